# revision 1
# baseline (speedup 1.0000x reference)
"""GNN message-passing layer for Trainium2, SPMD over 8 NeuronCores.

Computes, per batch b:
    XI = x @ Wi + b_msg                  # [N, F]
    XJ = x @ Wj                          # [N, F]
    agg[i, o] = sum_j adj[i, j] * relu(XI[i, o] + XJ[j, o])
    out = relu(x @ Wu1 + agg @ Wu2 + b_upd)

Sharding: core c handles batch b = c // 2 and i-rows [ih*256, ih*256+256),
ih = c % 2.  Each core gets full x (XJ needs all j), its i-slice of x and
adj, and replicated weights; outputs are the core's [256, 128] out-slice.

Per-core schedule (messages in bf16):
  - XI rows packed to one-partition-per-group layout; GPSIMD
    partition_broadcast materializes xib[j, (i,o)] per group of G i's.
  - DVE scalar_tensor_tensor adds XJ (free-dim step-0 repeat over i) + xib.
  - Relu split between DVE tensor_scalar (4x bf16) and ACT activation.
  - PE reduces over j: per (i, jt) matmul with relu'd messages [j, o] as
    stationary and the adj column [j, 1] as 1-wide moving operand,
    accumulating aggT[o, i] columns in PSUM across the 4 j-tiles.
  - Final linear fused as two f32 matmuls into one PSUM tile + ACT relu,
    PE transposes, DMA out.
"""

import numpy as np
from contextlib import ExitStack

import concourse.bass as bass
import concourse.mybir as mybir
import concourse.tile as tile
from concourse import bacc
from concourse.bass import ts
from concourse.bass_utils import run_bass_kernel_spmd
from concourse.masks import make_identity

B, N, F = 4, 512, 128
NCORES = 8
P = 128
IH = N // 2            # i-rows per core
NJT = N // P           # 4 j-tiles
NIT = IH // P          # 2 i-tiles
G = 32                 # i-group size for broadcast batching
NG = IH // G           # 16 groups
GF = G * F             # free size of one batched message tile
MSG_DT = mybir.dt.bfloat16
F32 = mybir.dt.float32
# relu engine assignment pattern, cycled per (group, j-tile) instruction:
# 'a' = ACT activation, 'v' = DVE tensor_scalar (4x bf16), 'g' = GPSIMD
RELU_PATTERN = "av"


def _kernel_body(ctx: ExitStack, tc: tile.TileContext, x, xh, adjh, w_msg,
                 b_msg, w_upd, b_upd, out, reps=1):
    nc = tc.nc
    RELU = mybir.ActivationFunctionType.Relu

    singles = ctx.enter_context(tc.tile_pool(name="singles", bufs=1))
    loads = ctx.enter_context(tc.tile_pool(name="loads", bufs=1))
    mpool = ctx.enter_context(tc.tile_pool(name="mpool", bufs=3))
    rpool = ctx.enter_context(tc.tile_pool(name="rpool", bufs=5))
    xibp = ctx.enter_context(tc.tile_pool(name="xibp", bufs=2))
    opool = ctx.enter_context(tc.tile_pool(name="opool", bufs=2))
    ppool = ctx.enter_context(tc.tile_pool(name="ppool", bufs=2, space="PSUM"))
    pagg = ctx.enter_context(tc.tile_pool(name="pagg", bufs=2, space="PSUM"))

    # ---- constants / weights -------------------------------------------
    identity = singles.tile([P, P], F32)
    make_identity(nc, identity)
    ones1 = singles.tile([1, P], F32)
    nc.vector.memset(ones1, 1.0)

    wi_sb = singles.tile([P, F], F32)
    nc.sync.dma_start(out=wi_sb, in_=w_msg[0:F, :])
    wj_sb = singles.tile([P, F], F32)
    nc.sync.dma_start(out=wj_sb, in_=w_msg[F:2 * F, :])
    wu1_sb = singles.tile([P, F], F32)
    nc.sync.dma_start(out=wu1_sb, in_=w_upd[0:F, :])
    wu2_sb = singles.tile([P, F], F32)
    nc.sync.dma_start(out=wu2_sb, in_=w_upd[F:2 * F, :])
    bmsg_sb = singles.tile([1, F], F32)
    nc.sync.dma_start(out=bmsg_sb, in_=b_msg[:, :])
    bupd_sb = singles.tile([P, 1], F32)
    nc.sync.dma_start(out=bupd_sb, in_=b_upd[:, :])

    # ---- load x / xh / adjh --------------------------------------------
    x_sb = loads.tile([P, N // P, F], F32)
    nc.sync.dma_start(out=x_sb, in_=x.rearrange("(t p) f -> p t f", p=P))
    xh_sb = loads.tile([P, NIT, F], F32)
    nc.sync.dma_start(out=xh_sb, in_=xh.rearrange("(t p) f -> p t f", p=P))
    adjh_sb = loads.tile([P, NIT, N], F32)
    nc.sync.dma_start(out=adjh_sb, in_=adjh.rearrange("(t p) j -> p t j", p=P))

    # ---- transposes: xbT [f, n=512], xhT [f, i=256] --------------------
    xbT = singles.tile([P, N], F32)
    for t in range(N // P):
        ps = ppool.tile([P, P], F32, tag="tp")
        nc.tensor.transpose(ps[:], x_sb[:, t, :], identity[:])
        nc.scalar.copy(out=xbT[:, ts(t, P)], in_=ps[:])
    xhT = singles.tile([P, IH], F32)
    for t in range(NIT):
        ps = ppool.tile([P, P], F32, tag="tp")
        nc.tensor.transpose(ps[:], xh_sb[:, t, :], identity[:])
        nc.scalar.copy(out=xhT[:, ts(t, P)], in_=ps[:])

    # ---- adjT [j, (jt, i)] bf16 ----------------------------------------
    adjT = singles.tile([P, NJT, IH], MSG_DT)
    for it in range(NIT):
        for jt in range(NJT):
            ps = ppool.tile([P, P], F32, tag="tp")
            nc.tensor.transpose(ps[:], adjh_sb[:, it, ts(jt, P)], identity[:])
            nc.scalar.copy(out=adjT[:, jt, ts(it, P)], in_=ps[:])

    # ---- XJ [j, o] bf16 tiles; XI [i, o] bf16 --------------------------
    xj_sb = singles.tile([P, NJT, F], MSG_DT)
    for t in range(NJT):
        ps = ppool.tile([P, F], F32, tag="mm")
        nc.tensor.matmul(ps[:], lhsT=xbT[:, ts(t, P)], rhs=wj_sb[:],
                         start=True, stop=True)
        nc.scalar.copy(out=xj_sb[:, t, :], in_=ps[:])
    xi_sb = singles.tile([P, NIT, F], MSG_DT)
    for t in range(NIT):
        ps = ppool.tile([P, F], F32, tag="mm")
        nc.tensor.matmul(ps[:], lhsT=xhT[:, ts(t, P)], rhs=wi_sb[:],
                         start=True, stop=False)
        nc.tensor.matmul(ps[:], lhsT=ones1[0:1, :], rhs=bmsg_sb[0:1, :],
                         start=False, stop=True)
        nc.scalar.copy(out=xi_sb[:, t, :], in_=ps[:])

    # ---- pack XI rows into per-group partitions: xi_lay[g, (s, o)] -----
    # pack XI rows to partition 0: dst free order == (row, o) == src
    # partition-major flatten, so one DMA per source i-tile suffices
    xi_lay = singles.tile([1, NG * GF], MSG_DT)
    for t in range(NIT):
        nc.sync.dma_start(
            out=xi_lay[0:1, t * P * F:(t + 1) * P * F],
            in_=xi_sb[:, t, :],
        )

    # ---- main loop ------------------------------------------------------
    for _rep in range(reps):
        _main_loop(nc, tc, mpool, rpool, xibp, opool, ppool, pagg, xi_lay,
                   xj_sb, adjT, xhT, wu1_sb, wu2_sb, bupd_sb, identity, out)


def _main_loop(nc, tc, mpool, rpool, xibp, opool, ppool, pagg, xi_lay,
               xj_sb, adjT, xhT, wu1_sb, wu2_sb, bupd_sb, identity, out):
    RELU = mybir.ActivationFunctionType.Relu
    paggT = pagg.tile([P, IH], F32)   # aggT[o, i] accumulator
    k = 0
    for g in range(NG):
        xib = xibp.tile([P, GF], MSG_DT)
        nc.gpsimd.partition_broadcast(xib[:, :],
                                      xi_lay[0:1, g * GF:(g + 1) * GF],
                                      channels=P)
        xib3 = xib[:].rearrange("p (s f) -> p s f", f=F)
        mrelus = []
        for jt in range(NJT):
            xj_ap = xj_sb[:, jt, :]
            xj_rep = bass.AP(tensor=xj_ap.tensor, offset=xj_ap.offset,
                             ap=[xj_ap.ap[0], [0, G], xj_ap.ap[1]])
            msum = mpool.tile([P, G, F], MSG_DT)
            nc.vector.tensor_tensor(
                out=msum[:, :, :], in0=xj_rep, in1=xib3,
                op=mybir.AluOpType.add)
            mrelu = rpool.tile([P, G, F], MSG_DT)
            eng = RELU_PATTERN[k % len(RELU_PATTERN)]
            if eng == "a":
                nc.scalar.activation(mrelu[:, :, :], msum[:, :, :], RELU)
            elif eng == "g":
                nc.gpsimd.tensor_scalar_max(mrelu[:, :, :], msum[:, :, :],
                                            0.0)
            else:
                nc.vector.tensor_scalar_max(mrelu[:, :, :], msum[:, :, :],
                                            0.0)
            k += 1
            mrelus.append(mrelu)
        for s in range(G):
            iloc = g * G + s
            for jt in range(NJT):
                nc.tensor.matmul(
                    paggT[:, iloc:iloc + 1],
                    lhsT=mrelus[jt][:, s, :],
                    rhs=adjT[:, jt, iloc:iloc + 1],
                    start=(jt == 0), stop=(jt == NJT - 1))

    # ---- epilogue: z = relu(x@Wu1 + agg@Wu2 + b_upd) -------------------
    aggT_sb = opool.tile([P, IH], F32)
    nc.scalar.copy(out=aggT_sb[:, :], in_=paggT[:, :])
    pz = pagg.tile([P, IH], F32)
    nc.tensor.matmul(pz[:], lhsT=wu1_sb[:], rhs=xhT[:, :],
                     start=True, stop=False)
    nc.tensor.matmul(pz[:], lhsT=wu2_sb[:], rhs=aggT_sb[:, :],
                     start=False, stop=True)
    zr = opool.tile([P, IH], F32)
    nc.scalar.activation(zr[:, :], pz[:, :], RELU, bias=bupd_sb[:, 0:1])

    out_sb = opool.tile([P, NIT, F], F32)
    for it in range(NIT):
        ps = ppool.tile([P, P], F32, tag="tp")
        nc.tensor.transpose(ps[:], zr[:, ts(it, P)], identity[:])
        nc.scalar.copy(out=out_sb[:, it, :], in_=ps[:])
    nc.sync.dma_start(out=out.rearrange("(t p) f -> p t f", p=P), in_=out_sb)


def build_nc(reps=1) -> bass.Bass:
    nc = bacc.Bacc("TRN2", target_bir_lowering=False, debug=False,
                   num_devices=NCORES)
    x = nc.dram_tensor("x", [N, F], F32, kind="ExternalInput")
    xh = nc.dram_tensor("xh", [IH, F], F32, kind="ExternalInput")
    adjh = nc.dram_tensor("adjh", [IH, N], F32, kind="ExternalInput")
    w_msg = nc.dram_tensor("w_msg", [2 * F, F], F32, kind="ExternalInput")
    b_msg = nc.dram_tensor("b_msg", [1, F], F32, kind="ExternalInput")
    w_upd = nc.dram_tensor("w_upd", [2 * F, F], F32, kind="ExternalInput")
    b_upd = nc.dram_tensor("b_upd", [F, 1], F32, kind="ExternalInput")
    out = nc.dram_tensor("out", [IH, F], F32, kind="ExternalOutput")
    with tile.TileContext(nc) as tc, ExitStack() as ctx:
        _kernel_body(ctx, tc, x[:], xh[:], adjh[:], w_msg[:], b_msg[:],
                     w_upd[:], b_upd[:], out[:], reps=reps)
    nc.compile()
    return nc


def make_in_maps(x, adj, W_msg, b_msg, W_upd, b_upd):
    in_maps = []
    for c in range(NCORES):
        b, ih = c // 2, c % 2
        sl = slice(ih * IH, (ih + 1) * IH)
        in_maps.append({
            "x": np.ascontiguousarray(x[b]),
            "xh": np.ascontiguousarray(x[b, sl]),
            "adjh": np.ascontiguousarray(adj[b, sl]),
            "w_msg": np.ascontiguousarray(W_msg),
            "b_msg": np.ascontiguousarray(b_msg.reshape(1, F)),
            "w_upd": np.ascontiguousarray(W_upd),
            "b_upd": np.ascontiguousarray(b_upd.reshape(F, 1)),
        })
    return in_maps


_NC_CACHE = None


def kernel(x, adj, W_msg, b_msg, W_upd, b_upd, _trace=False):
    global _NC_CACHE
    x = np.asarray(x, dtype=np.float32)
    adj = np.asarray(adj, dtype=np.float32)
    in_maps = make_in_maps(x, adj, np.asarray(W_msg, np.float32),
                           np.asarray(b_msg, np.float32),
                           np.asarray(W_upd, np.float32),
                           np.asarray(b_upd, np.float32))
    if _NC_CACHE is None:
        _NC_CACHE = build_nc()
    res = run_bass_kernel_spmd(_NC_CACHE, in_maps,
                               core_ids=list(range(NCORES)), trace=_trace)
    out = np.empty((B, N, F), dtype=np.float32)
    for c in range(NCORES):
        b, ih = c // 2, c % 2
        out[b, ih * IH:(ih + 1) * IH] = res.results[c]["out"]
    if _trace:
        kernel.last_results = res
    return out



# revision 17
# speedup vs baseline: 15.8449x; 15.8449x over previous
"""GNN message-passing layer for Trainium2, SPMD over 8 NeuronCores.

Computes, per batch b:
    XI = xh @ Wi + b_msg                 # [IH, F]   (i-half rows)
    XJ = x @ Wj                          # [N, F]
    agg[i, o] = sum_j adj[i, j] * relu(XI[i, o] + XJ[j, o])
    out = relu(xh @ Wu1 + agg @ Wu2 + b_upd)

Sharding: core c handles batch b = c // 2 and i-rows [ih*256, ih*256+256).

Per-core schedule ("delta-bank" design):
  Masking is folded additively: aBig = (adj - 1) * 2^30, so
  adj*relu(z) == relu(z + aBig) exactly (adj is 0/1).

  - o < OA ("PSUM path"): PE maintains per-(it, o%2) PSUM banks holding
    bank[i, j] = XJ[j, o] + aBig[i, j], advanced per o by a rank-1 update
    (ones_i (x) (XJ[.,o]-XJ[.,o-2])).  ACT consumes each bank with one
    activation: relu(bank + XI[:,o]) + free-dim accum -> agg column.
  - o >= OA ("SBUF path"): XJ row o is DMA-replicated across partitions
    (xjb); DVE tensor_tensor adds aBig (2x bf16); then one 4x tensor_scalar
    max(m, -XI[:,o]) with accum_out gives sum_j max(XJ+aBig, -XI), split
    DVE/GPSIMD.  Identity: agg = 512*XI + sum_j max(XJ + aBig, -XI), so the
    epilogue adds 512*XI for these columns only.
  - Epilogue: agg = agg_a + agg_v + agg_g + corr; PE transposes agg;
    out = relu(xh@Wu1 + agg@Wu2 + b_upd) via 3 accumulated matmuls + ACT.
"""

import numpy as np
from contextlib import ExitStack

import concourse.bass as bass
import concourse.mybir as mybir
import concourse.tile as tile
from concourse import bacc
from concourse.bass import ts
from concourse.bass_utils import run_bass_kernel_spmd
from concourse.masks import make_identity

B, N, F = 4, 512, 128
NCORES = 8
P = 128
IH = N // 2
NIT = IH // P          # 2 i-tiles
NPAR = 2               # psum bank parity (delta stride)
OA = 44                # o < OA: ACT/PSUM units; o >= OA: SBUF units
GP_S1 = 32             # of the (128-OA) sbuf o-slots, this many stage1s on GPSIMD
                       # (HW codegen only allows tensor_tensor on Pool, not
                       # tensor_scalar, so GPSIMD does stage1 merges)
K_O = 8                # o-rows per broadcast DMA block
BIGM = float(2 ** 30)
F32 = mybir.dt.float32
BF16 = mybir.dt.bfloat16
ALU = mybir.AluOpType
RELU = mybir.ActivationFunctionType.Relu


def _prelude(nc, singles, ppool, pwide, x, xh, adjh, w_msg, b_msg, w_upd,
             b_upd, xjstage):
    """Load inputs, build transposes and derived tensors (input-dependent,
    outside the reps loop)."""
    t = {}
    t["xjstage"] = xjstage
    t["identity"] = singles.tile([P, P], F32, name="identity")
    make_identity(nc, t["identity"])
    t["identity_bf"] = singles.tile([P, P], BF16, name="identity_bf")
    make_identity(nc, t["identity_bf"])
    t["ones1"] = singles.tile([1, P], F32, name="ones1")
    nc.vector.memset(t["ones1"], 1.0)
    t["onesP"] = singles.tile([P, P], BF16, name="onesP")
    nc.vector.memset(t["onesP"], 1.0)

    x_sb = singles.tile([P, N // P, F], F32)
    nc.sync.dma_start(out=x_sb, in_=x.rearrange("(t p) f -> p t f", p=P))
    for name, src in [("wj", w_msg[F:2 * F, :]), ("wi", w_msg[0:F, :])]:
        t[name] = singles.tile([P, F], F32, name=name)
        nc.sync.dma_start(out=t[name], in_=src)
    adjh_sb = singles.tile([P, NIT, N], F32)
    nc.sync.dma_start(out=adjh_sb, in_=adjh.rearrange("(t p) j -> p t j", p=P))
    xh_sb = singles.tile([P, NIT, F], F32)
    nc.sync.dma_start(out=xh_sb, in_=xh.rearrange("(t p) f -> p t f", p=P))
    for name, src in [("wu1", w_upd[0:F, :]), ("wu2", w_upd[F:2 * F, :])]:
        t[name] = singles.tile([P, F], F32, name=name)
        nc.sync.dma_start(out=t[name], in_=src)
    t["bmsg"] = singles.tile([1, F], F32, name="bmsg")
    nc.sync.dma_start(out=t["bmsg"], in_=b_msg[:, :])
    t["bupd"] = singles.tile([1, F], F32, name="bupd")
    nc.sync.dma_start(out=t["bupd"], in_=b_upd[:, :])

    # xT [f, n], xhT [f, i]
    t["xT"] = singles.tile([P, N], F32, name="xT")
    for k in range(N // P):
        ps = ppool.tile([P, P], F32, tag="tp")
        nc.tensor.transpose(ps[:], x_sb[:, k, :], t["identity"][:])
        nc.scalar.copy(out=t["xT"][:, ts(k, P)], in_=ps[:])
    t["xhT"] = singles.tile([P, IH], F32, name="xhT")
    for k in range(NIT):
        ps = ppool.tile([P, P], F32, tag="tp")
        nc.tensor.transpose(ps[:], xh_sb[:, k, :], t["identity"][:])
        nc.scalar.copy(out=t["xhT"][:, ts(k, P)], in_=ps[:])

    # XJT[o, j] = sum_f Wj[f, o] * xT[f, j]
    ps = pwide.tile([P, N], F32, tag="wide")
    nc.tensor.matmul(ps[:], lhsT=t["wj"][:], rhs=t["xT"][:], start=True,
                     stop=True)
    t["xjt"] = singles.tile([P, N], F32, name="xjt")
    nc.scalar.copy(out=t["xjt"], in_=ps[:])
    t["xjt_bf"] = singles.tile([P, N], BF16, name="xjt_bf")
    nc.vector.tensor_copy(out=t["xjt_bf"][:], in_=t["xjt"][:])

    # XI[i, (it, o)] = xh @ Wi + b_msg
    ps = pwide.tile([P, NIT * F], F32, tag="wide")
    for it in range(NIT):
        nc.tensor.matmul(ps[:, ts(it, F)], lhsT=t["xhT"][:, ts(it, P)],
                         rhs=t["wi"][:], start=True, stop=False)
        nc.tensor.matmul(ps[:, ts(it, F)], lhsT=t["ones1"][0:1, :],
                         rhs=t["bmsg"][0:1, :], start=False, stop=True)
    t["xi"] = singles.tile([P, NIT, F], F32, name="xi")
    nc.scalar.copy(out=t["xi"][:, :, :], in_=ps[:])
    t["negxi"] = singles.tile([P, NIT, F], F32, name="negxi")
    nc.vector.tensor_scalar(out=t["negxi"][:, :, :], in0=t["xi"][:, :, :],
                            scalar1=-1.0, scalar2=None, op0=ALU.mult)

    # aBig = (adj - 1) * 2^30  in bf16 ({0, -2^30} exactly)
    t["abig"] = singles.tile([P, NIT, N], BF16, name="abig")
    nc.vector.tensor_scalar(out=t["abig"][:, :, :], in0=adjh_sb[:, :, :],
                            scalar1=-1.0, scalar2=BIGM, op0=ALU.add,
                            op1=ALU.mult)

    # Stage XJT (bf16) to DRAM so broadcasts can be batched K_O rows per
    # DMA with a replicated read AP (SBUF sources can't batch across
    # partitions).  The bank-build outer-product rhs also reads row o from
    # partition 0 of the broadcast buffer (legal matmul base).
    nc.sync.dma_start(out=t["xjstage"][:, :], in_=t["xjt_bf"][:, :])

    # Correction tensor: 512*XI for sbuf columns (o >= OA), 0 for ACT columns
    t["xicorr"] = singles.tile([P, NIT, F], F32, name="xicorr")
    nc.vector.memset(t["xicorr"][:, :, 0:OA], 0.0)
    nc.vector.tensor_scalar(out=t["xicorr"][:, :, OA:F],
                            in0=t["xi"][:, :, OA:F],
                            scalar1=float(N), scalar2=None, op0=ALU.mult)
    return t


def _unit_order():
    """Interleave ACT units (o < OA) and SBUF units (o >= OA) so engines
    overlap; returns list of ("a"|"s", o) with both it-halves per entry."""
    acts = list(range(OA))
    sbufs = list(range(OA, F))
    order = []
    na, ns = len(acts), len(sbufs)
    ia = isb = 0
    # Bresenham-style merge proportional to counts
    err = 0
    while ia < na or isb < ns:
        if isb >= ns or (ia < na and err >= 0):
            order.append(("a", acts[ia])); ia += 1; err -= ns
        else:
            order.append(("s", sbufs[isb])); isb += 1; err += na
    return order


def _main_loop(nc, tc, t, banks, pscr, xjbp, mp, scrp, agg_a, agg_v, agg_g,
               ppool, out):
    # Init agg accumulators: every column is later overwritten by exactly
    # one unit's accum_out, except agg_v doubles as the xicorr carrier
    # (so the epilogue needs one fewer add).  agg_v columns written by units
    # are overwritten, so seed ALL of agg_v with xicorr and let agg_a keep
    # the zero/xicorr split: simplest correct scheme: agg_a zeroed, agg_g
    # zeroed, agg_v seeded with xicorr (its unit-written columns get
    # overwritten, and unwritten ones must then hold xicorr -- but xicorr is
    # only nonzero on sbuf columns, all of which ARE written by some unit).
    # So instead seed agg_a (ACT columns, xicorr=0 there is preserved on
    # non-ACT columns) with xicorr.
    nc.vector.tensor_copy(out=agg_a[:, :, :], in_=t["xicorr"][:, :, :])
    nc.gpsimd.memset(agg_v[:, :, :], 0.0)
    nc.gpsimd.memset(agg_g[:, :, :], 0.0)

    n_gp = 0
    n_s2 = 0
    blocks = {}

    def get_block(o):
        base = (o // K_O) * K_O
        if base not in blocks:
            # Broadcast K_O XJT rows across all partitions in one DMA
            # (DRAM source, replicated read AP).
            nrow = min(K_O, F - base)
            xjb = xjbp.tile([P, K_O, N], BF16)
            sap = t["xjstage"][base:base + nrow, :]
            rep = bass.AP(tensor=sap.tensor, offset=sap.offset,
                          ap=[[0, P]] + sap.ap)
            nc.sync.dma_start(out=xjb[:, 0:nrow, :], in_=rep)
            blocks[base] = xjb
        return blocks[base], base

    for kind, o in _unit_order():
        xjb, base = get_block(o)
        if kind == "a":
            par = o % NPAR
            for it in range(NIT):
                bk = banks[it][par]
                nc.tensor.matmul(bk[:], lhsT=t["identity_bf"][:],
                                 rhs=t["abig"][:, it, :], start=True,
                                 stop=False)
                nc.tensor.matmul(bk[:], lhsT=t["onesP"][0:1, :],
                                 rhs=xjb[0:1, o - base, :],
                                 start=False, stop=True)
                nc.scalar.activation(pscr[:], bk[:], RELU,
                                     bias=t["xi"][:, it, o:o + 1],
                                     accum_out=agg_a[:, it, o:o + 1])
        else:
            # One fused stage1 for both it-halves: m[:, it, j] =
            # XJ[j, o] + aBig[it]  (xjb row repeated via 0-stride AP).
            # A quota of stage1s runs on GPSIMD (tensor_tensor is its only
            # HW-legal tensor op) to relieve DVE; all stage2s run on DVE.
            m = mp.tile([P, NIT, N], BF16)
            row = xjb[:, o - base, :]
            row_rep = bass.AP(tensor=row.tensor, offset=row.offset,
                              ap=[row.ap[0], [0, NIT], row.ap[1]])
            use_gp = (n_gp * (F - OA) < GP_S1 * (n_s2 + 1))
            n_s2 += 1
            if use_gp:
                n_gp += 1
                nc.gpsimd.tensor_tensor(out=m[:, :, :], in0=row_rep,
                                        in1=t["abig"][:, :, :], op=ALU.add)
            else:
                nc.vector.tensor_tensor(out=m[:, :, :], in0=row_rep,
                                        in1=t["abig"][:, :, :], op=ALU.add)
            for it in range(NIT):
                scr = scrp.tile([P, N], BF16, tag="v")
                nc.vector.tensor_scalar(
                    out=scr[:], in0=m[:, it, :],
                    scalar1=t["negxi"][:, it, o:o + 1], scalar2=None,
                    op0=ALU.max, op1=ALU.add,
                    accum_out=agg_v[:, it, o:o + 1])

    # ---- epilogue ------------------------------------------------------
    aggsum = mp.tile([P, NIT, F], F32, tag="aggsum")
    nc.vector.tensor_tensor(out=aggsum[:, :, :], in0=agg_a[:, :, :],
                            in1=agg_v[:, :, :], op=ALU.add)
    nc.vector.tensor_tensor(out=aggsum[:, :, :], in0=aggsum[:, :, :],
                            in1=agg_g[:, :, :], op=ALU.add)

    aggT = mp.tile([P, NIT, P], F32, tag="aggT")
    for it in range(NIT):
        ps = ppool.tile([P, P], F32, tag="tp")
        nc.tensor.transpose(ps[:], aggsum[:, it, :], t["identity"][:])
        nc.scalar.copy(out=aggT[:, it, :], in_=ps[:])

    out_sb = mp.tile([P, NIT, F], F32, tag="out")
    for it in range(NIT):
        ps = ppool.tile([P, F], F32, tag="tp")
        nc.tensor.matmul(ps[:], lhsT=t["xhT"][:, ts(it, P)], rhs=t["wu1"][:],
                         start=True, stop=False)
        nc.tensor.matmul(ps[:], lhsT=aggT[:, it, :], rhs=t["wu2"][:],
                         start=False, stop=False)
        nc.tensor.matmul(ps[:], lhsT=t["ones1"][0:1, :], rhs=t["bupd"][0:1, :],
                         start=False, stop=True)
        nc.scalar.activation(out_sb[:, it, :], ps[:], RELU)
    nc.sync.dma_start(out=out.rearrange("(t p) f -> p t f", p=P), in_=out_sb)


def _kernel_body(ctx, tc, x, xh, adjh, w_msg, b_msg, w_upd, b_upd, out,
                 reps=1):
    nc = tc.nc
    xjstage = nc.dram_tensor("xjstage", [P, N], BF16, kind="Internal")
    singles = ctx.enter_context(tc.tile_pool(name="singles", bufs=1))
    ppool = ctx.enter_context(tc.tile_pool(name="ppool", bufs=2, space="PSUM"))
    pwide = ctx.enter_context(tc.tile_pool(name="pwide", bufs=1, space="PSUM"))
    pbank = ctx.enter_context(tc.tile_pool(name="pbank", bufs=1, space="PSUM"))
    pscrp = ctx.enter_context(tc.tile_pool(name="pscr", bufs=1, space="PSUM"))
    xjbp = ctx.enter_context(tc.tile_pool(name="xjbp", bufs=4))
    mp = ctx.enter_context(tc.tile_pool(name="mp", bufs=3))
    scrp = ctx.enter_context(tc.tile_pool(name="scrp", bufs=2))
    aggp = ctx.enter_context(tc.tile_pool(name="aggp", bufs=1))

    t = _prelude(nc, singles, ppool, pwide, x, xh, adjh, w_msg, b_msg,
                 w_upd, b_upd, xjstage[:])

    banks = [[pbank.tile([P, N], F32, name=f"bank{it}_{par}",
                         tag=f"bank{it}_{par}")
              for par in range(NPAR)] for it in range(NIT)]
    pscr = pscrp.tile([P, N], F32, tag="pscr")
    agg_a = aggp.tile([P, NIT, F], F32, tag="agg_a")
    agg_v = aggp.tile([P, NIT, F], F32, tag="agg_v")
    agg_g = aggp.tile([P, NIT, F], F32, tag="agg_g")

    for _rep in range(reps):
        _main_loop(nc, tc, t, banks, pscr, xjbp, mp, scrp, agg_a, agg_v,
                   agg_g, ppool, out)


def build_nc(reps=1) -> bass.Bass:
    nc = bacc.Bacc("TRN2", target_bir_lowering=False, debug=False,
                   num_devices=NCORES)
    x = nc.dram_tensor("x", [N, F], F32, kind="ExternalInput")
    xh = nc.dram_tensor("xh", [IH, F], F32, kind="ExternalInput")
    adjh = nc.dram_tensor("adjh", [IH, N], F32, kind="ExternalInput")
    w_msg = nc.dram_tensor("w_msg", [2 * F, F], F32, kind="ExternalInput")
    b_msg = nc.dram_tensor("b_msg", [1, F], F32, kind="ExternalInput")
    w_upd = nc.dram_tensor("w_upd", [2 * F, F], F32, kind="ExternalInput")
    b_upd = nc.dram_tensor("b_upd", [1, F], F32, kind="ExternalInput")
    out = nc.dram_tensor("out", [IH, F], F32, kind="ExternalOutput")
    with tile.TileContext(nc) as tc, ExitStack() as ctx:
        _kernel_body(ctx, tc, x[:], xh[:], adjh[:], w_msg[:], b_msg[:],
                     w_upd[:], b_upd[:], out[:], reps=reps)
    nc.compile()
    return nc


def make_in_maps(x, adj, W_msg, b_msg, W_upd, b_upd):
    in_maps = []
    for c in range(NCORES):
        b, ih = c // 2, c % 2
        sl = slice(ih * IH, (ih + 1) * IH)
        in_maps.append({
            "x": np.ascontiguousarray(x[b]),
            "xh": np.ascontiguousarray(x[b, sl]),
            "adjh": np.ascontiguousarray(adj[b, sl]),
            "w_msg": np.ascontiguousarray(W_msg),
            "b_msg": np.ascontiguousarray(b_msg.reshape(1, F)),
            "w_upd": np.ascontiguousarray(W_upd),
            "b_upd": np.ascontiguousarray(b_upd.reshape(1, F)),
        })
    return in_maps


_NC_CACHE = None


def kernel(x, adj, W_msg, b_msg, W_upd, b_upd, _trace=False):
    global _NC_CACHE
    x = np.asarray(x, dtype=np.float32)
    adj = np.asarray(adj, dtype=np.float32)
    in_maps = make_in_maps(x, adj, np.asarray(W_msg, np.float32),
                           np.asarray(b_msg, np.float32),
                           np.asarray(W_upd, np.float32),
                           np.asarray(b_upd, np.float32))
    if _NC_CACHE is None:
        _NC_CACHE = build_nc()
    res = run_bass_kernel_spmd(_NC_CACHE, in_maps,
                               core_ids=list(range(NCORES)), trace=_trace)
    out = np.empty((B, N, F), dtype=np.float32)
    for c in range(NCORES):
        b, ih = c // 2, c % 2
        out[b, ih * IH:(ih + 1) * IH] = res.results[c]["out"]
    if _trace:
        kernel.last_results = res
    return out


# revision 18
# speedup vs baseline: 21.4856x; 1.3560x over previous
"""GNN message-passing layer for Trainium2, SPMD over 8 NeuronCores.

Computes, per batch b:
    XI = xh @ Wi + b_msg                 # [IH, F]   (i-half rows)
    XJ = x @ Wj                          # [N, F]
    agg[i, o] = sum_j adj[i, j] * relu(XI[i, o] + XJ[j, o])
    out = relu(xh @ Wu1 + agg @ Wu2 + b_upd)

Sharding: core c handles batch b = c // 2 and i-rows [ih*256, ih*256+256).

Per-core schedule ("delta-bank" design):
  Masking is folded additively: aBig = (adj - 1) * 2^30, so
  adj*relu(z) == relu(z + aBig) exactly (adj is 0/1).

  - o < OA ("PSUM path"): PE maintains per-(it, o%2) PSUM banks holding
    bank[i, j] = XJ[j, o] + aBig[i, j], advanced per o by a rank-1 update
    (ones_i (x) (XJ[.,o]-XJ[.,o-2])).  ACT consumes each bank with one
    activation: relu(bank + XI[:,o]) + free-dim accum -> agg column.
  - o >= OA ("SBUF path"): XJ row o is DMA-replicated across partitions
    (xjb); DVE tensor_tensor adds aBig (2x bf16); then one 4x tensor_scalar
    max(m, -XI[:,o]) with accum_out gives sum_j max(XJ+aBig, -XI), split
    DVE/GPSIMD.  Identity: agg = 512*XI + sum_j max(XJ + aBig, -XI), so the
    epilogue adds 512*XI for these columns only.
  - Epilogue: agg = agg_a + agg_v + agg_g + corr; PE transposes agg;
    out = relu(xh@Wu1 + agg@Wu2 + b_upd) via 3 accumulated matmuls + ACT.
"""

import numpy as np
from contextlib import ExitStack

import concourse.bass as bass
import concourse.mybir as mybir
import concourse.tile as tile
from concourse import bacc
from concourse.bass import ts
from concourse.bass_utils import run_bass_kernel_spmd
from concourse.masks import make_identity

B, N, F = 4, 512, 128
NCORES = 8
P = 128
IH = N // 2
NIT = IH // P          # 2 i-tiles
NPAR = 2               # psum bank parity (delta stride)
OA = 44                # o < OA: ACT/PSUM units; o >= OA: SBUF units
GP_S1 = 32             # of the (128-OA) sbuf o-slots, this many stage1s on GPSIMD
                       # (HW codegen only allows tensor_tensor on Pool, not
                       # tensor_scalar, so GPSIMD does stage1 merges)
K_O = 8                # o-rows per broadcast DMA block
BIGM = float(2 ** 30)
F32 = mybir.dt.float32
BF16 = mybir.dt.bfloat16
ALU = mybir.AluOpType
RELU = mybir.ActivationFunctionType.Relu


def _prelude(nc, singles, ppool, pwide, x, xh, adjh, w_msg, b_msg, w_upd,
             b_upd, xjstage):
    """Load inputs, build transposes and derived tensors (input-dependent,
    outside the reps loop)."""
    t = {}
    t["xjstage"] = xjstage
    t["identity"] = singles.tile([P, P], F32, name="identity")
    make_identity(nc, t["identity"])
    t["identity_bf"] = singles.tile([P, P], BF16, name="identity_bf")
    make_identity(nc, t["identity_bf"])
    t["ones1"] = singles.tile([1, P], F32, name="ones1")
    nc.vector.memset(t["ones1"], 1.0)
    t["onesP"] = singles.tile([P, P], BF16, name="onesP")
    nc.vector.memset(t["onesP"], 1.0)

    x_sb = singles.tile([P, N // P, F], F32)
    nc.sync.dma_start(out=x_sb, in_=x.rearrange("(t p) f -> p t f", p=P))
    for name, src in [("wj", w_msg[F:2 * F, :]), ("wi", w_msg[0:F, :])]:
        t[name] = singles.tile([P, F], F32, name=name)
        nc.sync.dma_start(out=t[name], in_=src)
    adjh_sb = singles.tile([P, NIT, N], F32)
    nc.sync.dma_start(out=adjh_sb, in_=adjh.rearrange("(t p) j -> p t j", p=P))
    xh_sb = singles.tile([P, NIT, F], F32)
    nc.sync.dma_start(out=xh_sb, in_=xh.rearrange("(t p) f -> p t f", p=P))
    for name, src in [("wu1", w_upd[0:F, :]), ("wu2", w_upd[F:2 * F, :])]:
        t[name] = singles.tile([P, F], F32, name=name)
        nc.sync.dma_start(out=t[name], in_=src)
    t["bmsg"] = singles.tile([1, F], F32, name="bmsg")
    nc.sync.dma_start(out=t["bmsg"], in_=b_msg[:, :])
    t["bupd"] = singles.tile([1, F], F32, name="bupd")
    nc.sync.dma_start(out=t["bupd"], in_=b_upd[:, :])

    # xT [f, n], xhT [f, i]
    t["xT"] = singles.tile([P, N], F32, name="xT")
    for k in range(N // P):
        ps = ppool.tile([P, P], F32, tag="tp")
        nc.tensor.transpose(ps[:], x_sb[:, k, :], t["identity"][:])
        nc.scalar.copy(out=t["xT"][:, ts(k, P)], in_=ps[:])
    t["xhT"] = singles.tile([P, IH], F32, name="xhT")
    for k in range(NIT):
        ps = ppool.tile([P, P], F32, tag="tp")
        nc.tensor.transpose(ps[:], xh_sb[:, k, :], t["identity"][:])
        nc.scalar.copy(out=t["xhT"][:, ts(k, P)], in_=ps[:])

    # XJT[o, j] = sum_f Wj[f, o] * xT[f, j]
    ps = pwide.tile([P, N], F32, tag="wide")
    nc.tensor.matmul(ps[:], lhsT=t["wj"][:], rhs=t["xT"][:], start=True,
                     stop=True)
    t["xjt"] = singles.tile([P, N], F32, name="xjt")
    nc.scalar.copy(out=t["xjt"], in_=ps[:])
    t["xjt_bf"] = singles.tile([P, N], BF16, name="xjt_bf")
    nc.vector.tensor_copy(out=t["xjt_bf"][:], in_=t["xjt"][:])

    # XI[i, (it, o)] = xh @ Wi + b_msg
    ps = pwide.tile([P, NIT * F], F32, tag="wide")
    for it in range(NIT):
        nc.tensor.matmul(ps[:, ts(it, F)], lhsT=t["xhT"][:, ts(it, P)],
                         rhs=t["wi"][:], start=True, stop=False)
        nc.tensor.matmul(ps[:, ts(it, F)], lhsT=t["ones1"][0:1, :],
                         rhs=t["bmsg"][0:1, :], start=False, stop=True)
    t["xi"] = singles.tile([P, NIT, F], F32, name="xi")
    nc.scalar.copy(out=t["xi"][:, :, :], in_=ps[:])
    t["negxi"] = singles.tile([P, NIT, F], F32, name="negxi")
    nc.vector.tensor_scalar(out=t["negxi"][:, :, :], in0=t["xi"][:, :, :],
                            scalar1=-1.0, scalar2=None, op0=ALU.mult)

    # aBig = (adj - 1) * 2^30  in bf16 ({0, -2^30} exactly)
    t["abig"] = singles.tile([P, NIT, N], BF16, name="abig")
    nc.vector.tensor_scalar(out=t["abig"][:, :, :], in0=adjh_sb[:, :, :],
                            scalar1=-1.0, scalar2=BIGM, op0=ALU.add,
                            op1=ALU.mult)

    # Stage XJT (bf16) to DRAM so broadcasts can be batched K_O rows per
    # DMA with a replicated read AP (SBUF sources can't batch across
    # partitions).  The bank-build outer-product rhs also reads row o from
    # partition 0 of the broadcast buffer (legal matmul base).
    nc.sync.dma_start(out=t["xjstage"][:, :], in_=t["xjt_bf"][:, :])

    # Correction tensor: 512*XI for sbuf columns (o >= OA), 0 for ACT columns
    t["xicorr"] = singles.tile([P, NIT, F], F32, name="xicorr")
    nc.vector.memset(t["xicorr"][:, :, 0:OA], 0.0)
    nc.vector.tensor_scalar(out=t["xicorr"][:, :, OA:F],
                            in0=t["xi"][:, :, OA:F],
                            scalar1=float(N), scalar2=None, op0=ALU.mult)
    return t


def _unit_order():
    """Interleave ACT units (o < OA) and SBUF units (o >= OA) so engines
    overlap; returns list of ("a"|"s", o) with both it-halves per entry."""
    acts = list(range(OA))
    sbufs = list(range(OA, F))
    order = []
    na, ns = len(acts), len(sbufs)
    ia = isb = 0
    # Bresenham-style merge proportional to counts
    err = 0
    while ia < na or isb < ns:
        if isb >= ns or (ia < na and err >= 0):
            order.append(("a", acts[ia])); ia += 1; err -= ns
        else:
            order.append(("s", sbufs[isb])); isb += 1; err += na
    return order


def _main_loop(nc, tc, t, banks, pscr, xjbp, mp, scrp, agg_a, agg_v, agg_g,
               ppool, out):
    # Init agg accumulators: every column is later overwritten by exactly
    # one unit's accum_out, except agg_v doubles as the xicorr carrier
    # (so the epilogue needs one fewer add).  agg_v columns written by units
    # are overwritten, so seed ALL of agg_v with xicorr and let agg_a keep
    # the zero/xicorr split: simplest correct scheme: agg_a zeroed, agg_g
    # zeroed, agg_v seeded with xicorr (its unit-written columns get
    # overwritten, and unwritten ones must then hold xicorr -- but xicorr is
    # only nonzero on sbuf columns, all of which ARE written by some unit).
    # So instead seed agg_a (ACT columns, xicorr=0 there is preserved on
    # non-ACT columns) with xicorr.
    nc.vector.tensor_copy(out=agg_a[:, :, :], in_=t["xicorr"][:, :, :])
    nc.gpsimd.memset(agg_v[:, :, :], 0.0)
    nc.gpsimd.memset(agg_g[:, :, :], 0.0)

    n_gp = 0
    n_s2 = 0
    blocks = {}

    def get_block(o):
        base = (o // K_O) * K_O
        if base not in blocks:
            # Broadcast K_O XJT rows across all partitions in one DMA
            # (DRAM source, replicated read AP).
            nrow = min(K_O, F - base)
            xjb = xjbp.tile([P, K_O, N], BF16)
            sap = t["xjstage"][base:base + nrow, :]
            rep = bass.AP(tensor=sap.tensor, offset=sap.offset,
                          ap=[[0, P]] + sap.ap)
            nc.sync.dma_start(out=xjb[:, 0:nrow, :], in_=rep)
            blocks[base] = xjb
        return blocks[base], base

    for kind, o in _unit_order():
        xjb, base = get_block(o)
        if kind == "a":
            par = o % NPAR
            for it in range(NIT):
                bk = banks[it][par]
                nc.tensor.matmul(bk[:], lhsT=t["identity_bf"][:],
                                 rhs=t["abig"][:, it, :], start=True,
                                 stop=False)
                nc.tensor.matmul(bk[:], lhsT=t["onesP"][0:1, :],
                                 rhs=xjb[0:1, o - base, :],
                                 start=False, stop=True)
                nc.scalar.activation(pscr[:], bk[:], RELU,
                                     bias=t["xi"][:, it, o:o + 1],
                                     accum_out=agg_a[:, it, o:o + 1])
        else:
            # One fused stage1 for both it-halves: m[:, it, j] =
            # XJ[j, o] + aBig[it]  (xjb row repeated via 0-stride AP).
            # A quota of stage1s runs on GPSIMD (tensor_tensor is its only
            # HW-legal tensor op) to relieve DVE; all stage2s run on DVE.
            m = mp.tile([P, NIT, N], BF16)
            row = xjb[:, o - base, :]
            row_rep = bass.AP(tensor=row.tensor, offset=row.offset,
                              ap=[row.ap[0], [0, NIT], row.ap[1]])
            use_gp = (n_gp * (F - OA) < GP_S1 * (n_s2 + 1))
            n_s2 += 1
            if use_gp:
                n_gp += 1
                nc.gpsimd.tensor_tensor(out=m[:, :, :], in0=row_rep,
                                        in1=t["abig"][:, :, :], op=ALU.add)
            else:
                nc.vector.tensor_tensor(out=m[:, :, :], in0=row_rep,
                                        in1=t["abig"][:, :, :], op=ALU.add)
            for it in range(NIT):
                scr = scrp.tile([P, N], BF16, tag="v")
                nc.vector.tensor_scalar(
                    out=scr[:], in0=m[:, it, :],
                    scalar1=t["negxi"][:, it, o:o + 1], scalar2=None,
                    op0=ALU.max, op1=ALU.add,
                    accum_out=agg_v[:, it, o:o + 1])

    # ---- epilogue ------------------------------------------------------
    aggsum = mp.tile([P, NIT, F], F32, tag="aggsum")
    nc.vector.tensor_tensor(out=aggsum[:, :, :], in0=agg_a[:, :, :],
                            in1=agg_v[:, :, :], op=ALU.add)
    nc.vector.tensor_tensor(out=aggsum[:, :, :], in0=aggsum[:, :, :],
                            in1=agg_g[:, :, :], op=ALU.add)

    aggT = mp.tile([P, NIT, P], F32, tag="aggT")
    for it in range(NIT):
        ps = ppool.tile([P, P], F32, tag="tp")
        nc.tensor.transpose(ps[:], aggsum[:, it, :], t["identity"][:])
        nc.scalar.copy(out=aggT[:, it, :], in_=ps[:])

    out_sb = mp.tile([P, NIT, F], F32, tag="out")
    for it in range(NIT):
        ps = ppool.tile([P, F], F32, tag="tp")
        nc.tensor.matmul(ps[:], lhsT=t["xhT"][:, ts(it, P)], rhs=t["wu1"][:],
                         start=True, stop=False)
        nc.tensor.matmul(ps[:], lhsT=aggT[:, it, :], rhs=t["wu2"][:],
                         start=False, stop=False)
        nc.tensor.matmul(ps[:], lhsT=t["ones1"][0:1, :], rhs=t["bupd"][0:1, :],
                         start=False, stop=True)
        nc.scalar.activation(out_sb[:, it, :], ps[:], RELU)
    nc.sync.dma_start(out=out.rearrange("(t p) f -> p t f", p=P), in_=out_sb)
    return out_sb


def _kernel_body(ctx, tc, x, xh, adjh, w_msg, b_msg, w_upd, b_upd, out,
                 reps=1):
    nc = tc.nc
    xjstage = nc.dram_tensor("xjstage", [P, N], BF16, kind="Internal")
    singles = ctx.enter_context(tc.tile_pool(name="singles", bufs=1))
    ppool = ctx.enter_context(tc.tile_pool(name="ppool", bufs=2, space="PSUM"))
    pwide = ctx.enter_context(tc.tile_pool(name="pwide", bufs=1, space="PSUM"))
    pbank = ctx.enter_context(tc.tile_pool(name="pbank", bufs=1, space="PSUM"))
    pscrp = ctx.enter_context(tc.tile_pool(name="pscr", bufs=1, space="PSUM"))
    xjbp = ctx.enter_context(tc.tile_pool(name="xjbp", bufs=4))
    mp = ctx.enter_context(tc.tile_pool(name="mp", bufs=3))
    scrp = ctx.enter_context(tc.tile_pool(name="scrp", bufs=2))
    aggp = ctx.enter_context(tc.tile_pool(name="aggp", bufs=1))

    t = _prelude(nc, singles, ppool, pwide, x, xh, adjh, w_msg, b_msg,
                 w_upd, b_upd, xjstage[:])

    banks = [[pbank.tile([P, N], F32, name=f"bank{it}_{par}",
                         tag=f"bank{it}_{par}")
              for par in range(NPAR)] for it in range(NIT)]
    pscr = pscrp.tile([P, N], F32, tag="pscr")
    agg_a = aggp.tile([P, NIT, F], F32, tag="agg_a")
    agg_v = aggp.tile([P, NIT, F], F32, tag="agg_v")
    agg_g = aggp.tile([P, NIT, F], F32, tag="agg_g")

    for _rep in range(reps):
        out_sb = _main_loop(nc, tc, t, banks, pscr, xjbp, mp, scrp, agg_a,
                            agg_v, agg_g, ppool, out)
        if reps > 1:
            # Zero-scaled feedback: value-preserving (x*0 + abig) but forces
            # a true data dependency between reps so the compiler cannot
            # dead-code-eliminate repeated iterations in timing NEFFs.
            nc.vector.scalar_tensor_tensor(
                out=t["abig"][0:1, 0, 0:1], in0=out_sb[0:1, 0, 0:1],
                scalar=0.0, in1=t["abig"][0:1, 0, 0:1],
                op0=ALU.mult, op1=ALU.add)


def build_nc(reps=1) -> bass.Bass:
    nc = bacc.Bacc("TRN2", target_bir_lowering=False, debug=False,
                   num_devices=NCORES)
    x = nc.dram_tensor("x", [N, F], F32, kind="ExternalInput")
    xh = nc.dram_tensor("xh", [IH, F], F32, kind="ExternalInput")
    adjh = nc.dram_tensor("adjh", [IH, N], F32, kind="ExternalInput")
    w_msg = nc.dram_tensor("w_msg", [2 * F, F], F32, kind="ExternalInput")
    b_msg = nc.dram_tensor("b_msg", [1, F], F32, kind="ExternalInput")
    w_upd = nc.dram_tensor("w_upd", [2 * F, F], F32, kind="ExternalInput")
    b_upd = nc.dram_tensor("b_upd", [1, F], F32, kind="ExternalInput")
    out = nc.dram_tensor("out", [IH, F], F32, kind="ExternalOutput")
    with tile.TileContext(nc) as tc, ExitStack() as ctx:
        _kernel_body(ctx, tc, x[:], xh[:], adjh[:], w_msg[:], b_msg[:],
                     w_upd[:], b_upd[:], out[:], reps=reps)
    nc.compile()
    return nc


def make_in_maps(x, adj, W_msg, b_msg, W_upd, b_upd):
    in_maps = []
    for c in range(NCORES):
        b, ih = c // 2, c % 2
        sl = slice(ih * IH, (ih + 1) * IH)
        in_maps.append({
            "x": np.ascontiguousarray(x[b]),
            "xh": np.ascontiguousarray(x[b, sl]),
            "adjh": np.ascontiguousarray(adj[b, sl]),
            "w_msg": np.ascontiguousarray(W_msg),
            "b_msg": np.ascontiguousarray(b_msg.reshape(1, F)),
            "w_upd": np.ascontiguousarray(W_upd),
            "b_upd": np.ascontiguousarray(b_upd.reshape(1, F)),
        })
    return in_maps


_NC_CACHE = None


def kernel(x, adj, W_msg, b_msg, W_upd, b_upd, _trace=False):
    global _NC_CACHE
    x = np.asarray(x, dtype=np.float32)
    adj = np.asarray(adj, dtype=np.float32)
    in_maps = make_in_maps(x, adj, np.asarray(W_msg, np.float32),
                           np.asarray(b_msg, np.float32),
                           np.asarray(W_upd, np.float32),
                           np.asarray(b_upd, np.float32))
    if _NC_CACHE is None:
        _NC_CACHE = build_nc()
    res = run_bass_kernel_spmd(_NC_CACHE, in_maps,
                               core_ids=list(range(NCORES)), trace=_trace)
    out = np.empty((B, N, F), dtype=np.float32)
    for c in range(NCORES):
        b, ih = c // 2, c % 2
        out[b, ih * IH:(ih + 1) * IH] = res.results[c]["out"]
    if _trace:
        kernel.last_results = res
    return out
